# revision 4
# baseline (speedup 1.0000x reference)
"""Trainium2 Bass kernel for nn_BasicBlock (ReActNet-style binary basic block).

Forward math (stop_gradient collapses in forward):
    s1   = sign(x + b11)                          in {-1, 0, +1}
    pre1 = A1*conv3x3(s1, sign(w1)) + K1 + x      [BN folded: A1=scale1*inv1,
                                                   K1=be1-m1*inv1+b12]
    p1   = prelu(pre1, a1) = a1h*pre1 + b1h*|pre1|
    s2   = sign(p1 + b13 + b21)
    pre2 = A2*conv1x1(s2, sign(w2)) + K2 + p1     [K2=be2-m2*inv2+b22+b13]
    out  = a2h*pre2 + b2h*|pre2| + b23

Both convs are +-1 matmuls: exact in fp8e4 with fp32 PSUM accumulation,
run in DoubleRow perf mode (2 K-tiles of 128 fused, 0.5 cycles/row).
Residual/bias folding into PSUM:
  - K1/A1 enters psum1 via a K=2 fp8 matmul (hi/lo split) against ones.
  - p1 enters psum2 via a diag(a1h/A2) fp16 matmul with rhs=inner1.
Epilogue runs on fp16 intermediates (rel err ~1.5e-3 vs f32 reference):
  P      = A1*psum1 + x                   (DVE stt, = pre1)
  Pabs   = |P|                            (DVE u16 bitand)
  inner1 = r1*Pabs + P                    (DVE stt, = p1/a1h, r1=b1h/a1h)
  s2     = Sign(a1h*inner1 + b21p)        (ACT, fp8)
  R      = A2*psum2 + K2                  (ACT, = pre2)
  Rabs   = |R|                            (DVE u16 bitand)
  inner2 = r2*Rabs + R                    (DVE stt)
  out    = a2h*inner2 + b23               (Pool ts, f32)
Sharding: data-parallel over batch, 16 images per core on 8 cores.
"""

import numpy as np
import ml_dtypes

import concourse.bacc as bacc
import concourse.mybir as mybir
from concourse import tile
from concourse.bass_utils import run_bass_kernel_spmd

N_CORES = 8
B, C, H, W = 128, 256, 32, 32
PIMG = B // N_CORES  # images per core
EPS = 1e-5

_CACHE = {}

# cv columns (per ct): 0 b11, 1 A1, 2 r1, 3 a1h, 4 b21p, 5 A2, 6 K2,
#                      7 r2, 8 a2h, 9 b23
NCV = 10


def _build_program(n_img):
    """Build the SPMD per-core Bass/Tile program (same program on all cores)."""
    f32, fp16, fp8 = mybir.dt.float32, mybir.dt.float16, mybir.dt.float8e4
    u16 = mybir.dt.uint16
    AF = mybir.ActivationFunctionType
    ALU = mybir.AluOpType
    DR = mybir.MatmulPerfMode.DoubleRow

    nc = bacc.Bacc("TRN2", target_bir_lowering=False, debug=False,
                   num_devices=N_CORES)

    x_d = nc.dram_tensor("x", [n_img, C, H, W], f32, kind="ExternalInput").ap()
    w1_d = nc.dram_tensor("w1t", [128, 9, 2, 2, 128], fp8,
                          kind="ExternalInput").ap()
    w2_d = nc.dram_tensor("w2t", [128, 2, 2, 128], fp8,
                          kind="ExternalInput").ap()
    d2_d = nc.dram_tensor("d2t", [128, 2, 128], fp16,
                          kind="ExternalInput").ap()
    b1_d = nc.dram_tensor("b1t", [2, 2, 128], fp8, kind="ExternalInput").ap()
    cv_d = nc.dram_tensor("cv", [2, 128, NCV], f32, kind="ExternalInput").ap()
    out_d = nc.dram_tensor("out", [n_img, C, H, W], f32,
                           kind="ExternalOutput").ap()

    with tile.TileContext(nc) as tc:
        with tc.tile_pool(name="wp", bufs=1) as wp, \
             tc.tile_pool(name="work", bufs=1) as work, \
             tc.tile_pool(name="ps", bufs=1, space="PSUM") as ps:

            # consts first (tiny, ONE dma) so sign1(0) starts while weights
            # stream
            cvt = wp.tile([128, 2, NCV], f32, name="cvt")
            nc.sync.dma_start(out=cvt, in_=cv_d.rearrange("t p c -> p t c"))
            cv = [cvt[:, 0], cvt[:, 1]]

            ones8 = wp.tile([2, 512], fp8, name="ones8")
            nc.vector.memset(ones8, 1.0)

            wsb = {}

            def load_weights():
                # emitted after stage_a(0): x(0)+sign1(0) overlap the weight
                # stream; per-oct split lets conv1(0) oct=0 start as soon as
                # its half of w1 lands
                w1sb = wp.tile([128, 9, 2, 2, 128], fp8, name="w1sb")
                for oc in range(2):
                    nc.sync.dma_start(out=w1sb[:, :, :, oc, :],
                                      in_=w1_d[:, :, :, oc, :])
                w2sb = wp.tile([128, 2, 2, 128], fp8, name="w2sb")
                nc.sync.dma_start(out=w2sb, in_=w2_d)
                d2sb = wp.tile([128, 2, 128], fp16, name="d2sb")
                nc.sync.dma_start(out=d2sb, in_=d2_d)
                b1sb = wp.tile([2, 2, 128], fp8, name="b1sb")
                nc.sync.dma_start(out=b1sb, in_=b1_d)
                wsb.update(w1=w1sb, w2=w2sb, d2=d2sb, b1=b1sb)

            xts, s1ps, Ps, in1s = {}, {}, {}, {}

            def stage_a(i):
                # load x(i); s1(i) = sign(x + b11) into padded fp8 tile
                # (borders pre-zeroed once per buffer at warmup)
                xts[i] = []
                sp = work.tile([128, 2, 34, 34], fp8, tag="s1p", bufs=3,
                               name=f"s1p_{i}")
                for ct in range(2):
                    xt = work.tile([128, H, W], f32, tag=f"xt{ct}", bufs=4,
                                   name=f"xt{ct}_{i}")
                    nc.sync.dma_start(out=xt,
                                      in_=x_d[i, ct * 128:(ct + 1) * 128])
                    nc.scalar.activation(sp[:, ct, 1:33, 1:33], xt, AF.Sign,
                                         bias=cv[ct][:, 0:1])
                    xts[i].append(xt)
                s1ps[i] = sp

            def stage_b(i):
                # conv1(i): 9 DR matmuls + bias matmul per (oct,hh);
                # P = A1*psum + x (DVE); inner1 = r1*|P| + P; s2 = Sign (ACT)
                sp = s1ps[i]
                Ps[i], in1s[i] = [], []
                s2p = work.tile([128, 2, 1024], fp8, tag="s2p", bufs=3,
                                name=f"s2p_{i}")
                for oct in range(2):
                    P = work.tile([128, 1024], fp16, tag=f"P{oct}", bufs=2,
                                  name=f"P{oct}_{i}")
                    xflat = xts[i][oct].rearrange("p a b -> p (a b)")
                    for hh in range(2):
                        p1t = ps.tile([128, 512], f32, tag=f"ps1_{oct}{hh}",
                                      bufs=1, name=f"ps1_{oct}{hh}_{i}")
                        for tap in range(9):
                            kh, kw = divmod(tap, 3)
                            nc.tensor.matmul(
                                p1t,
                                lhsT=wsb["w1"][:, tap, :, oct, :],
                                rhs=sp[:, :, hh * 16 + kh:hh * 16 + kh + 16,
                                       kw:kw + 32],
                                start=(tap == 0), stop=False, perf_mode=DR)
                        nc.tensor.matmul(p1t, lhsT=wsb["b1"][:, oct, :],
                                         rhs=ones8, start=False, stop=True)
                        nc.vector.scalar_tensor_tensor(
                            P[:, hh * 512:(hh + 1) * 512], p1t,
                            cv[oct][:, 1:2],
                            xflat[:, hh * 512:(hh + 1) * 512],
                            op0=ALU.mult, op1=ALU.add)
                    Pa = work.tile([128, 1024], fp16, tag=f"Pa{oct}", bufs=2,
                                   name=f"Pa{oct}_{i}")
                    nc.vector.tensor_scalar(Pa.bitcast(u16), P.bitcast(u16),
                                            0x7FFF, None, op0=ALU.bitwise_and)
                    in1 = work.tile([128, 1024], fp16, tag=f"in1{oct}",
                                    bufs=3, name=f"in1{oct}_{i}")
                    nc.vector.scalar_tensor_tensor(in1, Pa, cv[oct][:, 2:3],
                                                   P, op0=ALU.mult,
                                                   op1=ALU.add)
                    nc.scalar.activation(s2p[:, oct, :], in1, AF.Sign,
                                         bias=cv[oct][:, 4:5],
                                         scale=cv[oct][:, 3:4])
                    Ps[i].append(P)
                    in1s[i].append(in1)
                s1ps[i] = None
                s2ps[i] = s2p

            s2ps = {}

            def stage_c(i):
                # conv2(i) + diag residual; R = A2*psum + K2 (ACT);
                # Rabs (Pool); inner2 (DVE); out = a2h*inner2 + b23 (Pool)
                s2p = s2ps[i]
                for oct in range(2):
                    p2t = ps.tile([128, 1024], f32, tag=f"ps2_{oct}", bufs=1,
                                  name=f"ps2_{oct}_{i}")
                    for hh in range(2):
                        sl = slice(hh * 512, (hh + 1) * 512)
                        nc.tensor.matmul(p2t[:, sl],
                                         lhsT=wsb["w2"][:, :, oct, :],
                                         rhs=s2p[:, :, sl],
                                         start=True, stop=False, perf_mode=DR)
                        nc.tensor.matmul(p2t[:, sl],
                                         lhsT=wsb["d2"][:, oct, :],
                                         rhs=in1s[i][oct][:, sl],
                                         start=False, stop=True)
                    R = work.tile([128, 1024], fp16, tag=f"R{oct}", bufs=2,
                                  name=f"R{oct}_{i}")
                    nc.scalar.activation(R, p2t, AF.Identity,
                                         bias=cv[oct][:, 6:7],
                                         scale=cv[oct][:, 5:6])
                    Ra = work.tile([128, 1024], fp16, tag=f"Ra{oct}", bufs=2,
                                   name=f"Ra{oct}_{i}")
                    nc.vector.tensor_scalar(Ra.bitcast(u16), R.bitcast(u16),
                                            0x7FFF, None, op0=ALU.bitwise_and)
                    in2 = work.tile([128, 1024], fp16, tag=f"in2{oct}",
                                    bufs=2, name=f"in2{oct}_{i}")
                    nc.vector.scalar_tensor_tensor(in2, Ra, cv[oct][:, 7:8],
                                                   R, op0=ALU.mult,
                                                   op1=ALU.add)
                    of = work.tile([128, 1024], f32, tag=f"of{oct}", bufs=2,
                                   name=f"of{oct}_{i}")
                    nc.gpsimd.tensor_scalar(of, in2, cv[oct][:, 8:9],
                                            cv[oct][:, 9:10],
                                            op0=ALU.mult, op1=ALU.add)
                    nc.sync.dma_start(
                        out=out_d[i, oct * 128:(oct + 1) * 128],
                        in_=of.rearrange("p (a b) -> p a b", a=H))
                in1s[i] = None
                s2ps[i] = None
                Ps[i] = None

            # PE warm-up: dummy matmuls on a zeroed tile run during the
            # startup DMA wait so conv1(0) starts at full HAM clock rate
            warm = work.tile([128, 512], mybir.dt.bfloat16, name="warm")
            nc.gpsimd.memset(warm, 0.0)
            wps = ps.tile([128, 512], f32, tag="ps1_00", bufs=1, name="wps")
            for r in range(18):
                nc.tensor.matmul(wps, lhsT=warm[:, 0:128], rhs=warm,
                                 start=(r == 0), stop=(r == 17))

            # pre-zero the s1p borders once per rotating buffer
            for r in range(3):
                spz = work.tile([128, 2, 34, 34], fp8, tag="s1p", bufs=3,
                                name=f"s1pz_{r}")
                nc.gpsimd.memset(spz[:, :, 0, :], 0.0)
                nc.gpsimd.memset(spz[:, :, 33, :], 0.0)
                nc.gpsimd.memset(spz[:, :, 1:33, 0], 0.0)
                nc.gpsimd.memset(spz[:, :, 1:33, 33], 0.0)

            for it in range(n_img + 2):
                if it < n_img:
                    stage_a(it)
                if it == 0:
                    load_weights()
                if 1 <= it <= n_img:
                    stage_b(it - 1)
                if 2 <= it:
                    stage_c(it - 2)

    nc.compile()
    return nc


def _prep_host(inputs):
    """Host-side O(C^2) weight/constant preprocessing (numpy)."""
    f = lambda k: np.asarray(inputs[k], dtype=np.float32)
    fp8 = ml_dtypes.float8_e4m3
    w1, w2 = f("w1"), f("w2")
    b11, b12, b13 = f("b11"), f("b12"), f("b13")
    b21, b22, b23 = f("b21"), f("b22"), f("b23")
    a1, a2 = f("a1"), f("a2")
    g1, be1, m1, v1 = f("g1m"), f("be1m"), f("m1m"), f("v1m")
    g2, be2, m2, v2 = f("g2m"), f("be2m"), f("m2m"), f("v2m")

    scale1 = np.abs(w1).mean(axis=(1, 2, 3), dtype=np.float64).astype(np.float32)
    scale2 = np.abs(w2).mean(axis=(1, 2, 3), dtype=np.float64).astype(np.float32)

    # sign(w1): [oc,ic,kh,kw] -> [ic_lo, tap, ict, oct, oc_lo]
    sgn1 = np.sign(w1).reshape(2, 128, 2, 128, 9).transpose(3, 4, 2, 0, 1)
    w1t = np.ascontiguousarray(sgn1).astype(fp8)
    # sign(w2): [oc,ic] -> [ic_lo, ict, oct, oc_lo]
    sgn2 = np.sign(w2.reshape(256, 256)).reshape(2, 128, 2, 128)
    w2t = np.ascontiguousarray(sgn2.transpose(3, 2, 0, 1)).astype(fp8)

    inv1 = g1 / np.sqrt(v1 + EPS)
    inv2 = g2 / np.sqrt(v2 + EPS)
    A1 = scale1 * inv1
    K1 = be1 - m1 * inv1 + b12
    A2 = scale2 * inv2
    K2 = be2 - m2 * inv2 + b22 + b13
    a1h, b1h = (1.0 + a1) / 2.0, (1.0 - a1) / 2.0
    a2h, b2h = (1.0 + a2) / 2.0, (1.0 - a2) / 2.0

    # diag2: [row=128, oct, col=128] fp16, diag(a1h/A2) per oct
    d = (a1h / A2).astype(np.float16)
    d2t = np.zeros((128, 2, 128), np.float16)
    for o in range(2):
        d2t[np.arange(128), o, np.arange(128)] = d[o * 128:(o + 1) * 128]

    # bias1: K1/A1 split hi/lo in fp8: [2, oct, 128]
    c1 = K1 / A1
    c1hi = c1.astype(fp8)
    c1lo = (c1 - c1hi.astype(np.float32)).astype(fp8)
    b1t = np.stack([c1hi.reshape(2, 128), c1lo.reshape(2, 128)], axis=0)

    cv = np.stack([
        b11, A1, b1h / a1h, a1h, b13 + b21,
        A2, K2, b2h / a2h, a2h, b23,
    ], axis=-1).astype(np.float32).reshape(2, 128, NCV)
    return (w1t, w2t, np.ascontiguousarray(d2t),
            np.ascontiguousarray(b1t), np.ascontiguousarray(cv))


def _make_runner(nc):
    """Persistent jitted 8-core executor (compiles once, reusable across
    kernel() calls). Mirrors bass2jax.run_bass_via_pjrt's multi-core path."""
    import jax
    from jax.experimental.shard_map import shard_map
    from jax.sharding import Mesh, PartitionSpec
    from concourse.bass2jax import (install_neuronx_cc_hook, _bass_exec_p,
                                    partition_id_tensor)

    install_neuronx_cc_hook()
    pname = nc.partition_id_tensor.name if nc.partition_id_tensor else None
    in_names, out_names, out_avals, zero_outs = [], [], [], []
    for alloc in nc.m.functions[0].allocations:
        if not isinstance(alloc, mybir.MemoryLocationSet):
            continue
        name = alloc.memorylocations[0].name
        if alloc.kind == "ExternalInput":
            if name != pname:
                in_names.append(name)
        elif alloc.kind == "ExternalOutput":
            out_names.append(name)
            shape = tuple(alloc.tensor_shape)
            dtype = mybir.dt.np(alloc.dtype)
            out_avals.append(jax.core.ShapedArray(shape, dtype))
            zero_outs.append(np.zeros(shape, dtype))
    all_names = in_names + out_names + ([pname] if pname else [])

    def _body(*args):
        operands = list(args)
        if pname is not None:
            operands.append(partition_id_tensor())
        return tuple(_bass_exec_p.bind(
            *operands, out_avals=tuple(out_avals), in_names=tuple(all_names),
            out_names=tuple(out_names), lowering_input_output_aliases=(),
            sim_require_finite=True, sim_require_nnan=True, nc=nc))

    devices = jax.devices()[:N_CORES]
    assert len(devices) == N_CORES
    mesh = Mesh(np.asarray(devices), ("core",))
    spec = PartitionSpec("core")
    n_args = len(in_names) + len(out_names)
    jitted = jax.jit(
        shard_map(_body, mesh=mesh, in_specs=(spec,) * n_args,
                  out_specs=(spec,) * len(out_names), check_rep=False),
        keep_unused=True,
    )

    def run(per_core_in):
        concat_in = [np.concatenate([m[nm] for m in per_core_in], axis=0)
                     for nm in in_names]
        concat_zeros = [np.zeros((N_CORES * z.shape[0], *z.shape[1:]), z.dtype)
                        for z in zero_outs]
        outs = jitted(*concat_in, *concat_zeros)
        oix = out_names.index("out")
        return np.asarray(outs[oix])  # [N_CORES*PIMG, C, H, W]

    return run


def kernel(**inputs):
    x = np.ascontiguousarray(np.asarray(inputs["x"], dtype=np.float32))
    w1t, w2t, d2t, b1t, cv = _prep_host(inputs)

    if "nc" not in _CACHE:
        _CACHE["nc"] = _build_program(PIMG)
    nc = _CACHE["nc"]

    in_maps = [{
        "x": x[c * PIMG:(c + 1) * PIMG],
        "w1t": w1t,
        "w2t": w2t,
        "d2t": d2t,
        "b1t": b1t,
        "cv": cv,
    } for c in range(N_CORES)]

    try:
        if "runner" not in _CACHE:
            _CACHE["runner"] = _make_runner(nc)
        return _CACHE["runner"](in_maps)
    except Exception:
        _CACHE.pop("runner", None)
        res = run_bass_kernel_spmd(nc, in_maps, core_ids=list(range(N_CORES)))
        return np.concatenate([r["out"] for r in res.results], axis=0)
